# revision 36
# baseline (speedup 1.0000x reference)
"""Trainium2 Bass kernel for nn_KGEdges via low-rank trigonometric factorization.

S[b,i,j] = sum_d w[d] * tanh(h[b,j,d] + c[b,i,d]) + mm[b,i] + mm[b,j]
  with h = x@Wh.T + bh, c = x@Wc.T.

Math: tanh(x) ~= a1*x + sum_m b_m sin(om_m x) (weighted LSQ fit, rel err
~2e-3 under the data distribution), and each sine factorizes over h+c:
sin(om(h+c)) = sin(om h)cos(om c) + cos(om h)sin(om c). The (i,j) plane
then comes from ONE PE contraction over (feature, d):

  S[i,j] = sum_{f,d} Cf[f,d,i] * Hf[f,d,j]
  Hf: {1, h, sin(om_m h), cos(om_m h)}            (2M+2 features)
  Cf: {w*a1*c, 1, w*b_m cos(om_m c), w*b_m sin(om_m c)}
  (+ a 2-row mask chunk: [1;mm_i] x [mm_j;1])

Range reduction for Sin (HW spline valid on [-pi,pi] only) is done in
int16 fixed point: one fused DVE tensor_scalar per (freq, d-half) computes
k32 = round(v*(om*65536/2pi) + 2^20) -> i32; the *signed low halfword* of
k32 (i16 stride-2 bitcast view) IS the angle mod 2pi mapped onto
[-32768,32768) ~ [-pi,pi), consumed by giant Sin passes with
scale=2pi/65536.  cos slots are derived on the otherwise-idle Pool engine
as k_cos = k_sin + 16384 (pi/2 exact in fixed point; the low halfword
wraps mod 2^16 for free), halving the DVE slot work.

Perf structure (vs the 35.0us predecessor):
 - ~27 junk matmuls warm the PE HAM clock gate during the DMA-in window so
   every real matmul runs at 2.4GHz instead of the cold 1.2GHz.
 - input DMA split into ~128KB chunks on 4 queues (sync/scalar/gpsimd/
   vector) ordered x+Wh(dh0) -> Wh(dh1) -> Wc; weights packed dh-major so
   the dh0 projection (and the whole feature pipeline behind it) starts
   ~1us earlier.
 - output in bf16 over two queues (host upcasts), halving the tail DMA.
Data-parallel over batch: 1 core per batch element.
"""

import os
import sys

for _p in ("/opt/trn_rl_repo", "/opt/pypackages"):
    if _p not in sys.path and os.path.isdir(_p):
        sys.path.insert(0, _p)

import numpy as np

from concourse import bass, tile
import concourse.mybir as mybir
from concourse.bass_utils import run_bass_kernel_spmd

BS, SL, ENC, ED = 8, 256, 1024, 256
P = 128
KO = ENC // P      # 8 contraction chunks for projections
DH = ED // P       # 2 d-halves
FD = DH * SL       # 512: free size of one (d, s) plane per partition

# sine fit of tanh on N(0,sqrt2): tanh(x) ~= A1*x + sum b sin(om x)
OM = [0.8457601956781701, 1.7785856286560242, 2.9048194005148917]
BM = [0.4985272230495812, 0.1235633227644961, 0.026858281756312094]
M = len(OM)
A1 = 0.2644975130911238
TWO_PI = 2.0 * np.pi
PH_SCALE = 65536.0 / TWO_PI        # angle -> fixed-point units
ACT_SCALE = float(TWO_PI / 65536.0)
OFF = float(2 ** 20)               # keeps the TS output positive in f32
QUARTER = 16384.0                  # pi/2 in fixed-point units (2^16/4)
NSLOT = 2 * M                      # slots per side: sin m -> m, cos m -> M+m

# toggles for bisecting hardware failures
WARMUP_N = 40                      # junk matmuls to warm the PE clock gate
                                   # (>3.4us contiguous: the HAM window,
                                   # plus margin for window phase jitter)
WARM_FREE = 128
OUT_BF16 = True
# f32 mantissa trick: t = v*s1 + (3*2^22 + ph) pins the exponent so the f32
# mantissa's low halfword IS round(v*s1+ph) mod 2^16 -- the same angle the
# i32-convert path produced, but as a pure f32 single-src op (DVE 2x-mode
# eligible, no int convert).
MAGIC = float(3 * 2 ** 22)

# f32 tail param columns (per-partition vectors)
T_BH = 0                    # bh by d-half                     (DH cols)
T_WB = T_BH + DH            # w*b_m by (m, dh)                 (M*DH cols)
T_WA1 = T_WB + M * DH       # w*a1 by dh                       (DH cols)
T_TOT = T_WA1 + DH

# packed bf16 input: [xS | tail(f32->bf16 pairs) | WcS | WhS]; the tail rides
# inside the x ko6-7 DMA chunk so it needs no DMA (and receipt) of its own
F_SEC = KO * SL
OFF_X = 0
OFF_TAIL = F_SEC
OFF_WC = F_SEC + 2 * T_TOT
OFF_WH = OFF_WC + F_SEC
F_PRJ = OFF_WH + F_SEC

F32 = mybir.dt.float32
F16 = mybir.dt.float16
BF16 = mybir.dt.bfloat16
I32 = mybir.dt.int32
I16 = mybir.dt.int16
AF = mybir.ActivationFunctionType
ALU = mybir.AluOpType

_CACHE: dict = {}

_ENGINE_SEM_PREFIXES = ("Activation", "DVE", "PE", "Pool", "SP", "DMAHW", "DMASW")


def _strip_self_waits(raw: bytes) -> bytes:
    """Remove provably-satisfied self-engine semaphore waits; split residual
    multi-waits on operand-free sync instructions (walrus encodes at most one
    sync wait per instruction)."""
    import json

    m = json.loads(raw)
    for fn in m["functions"]:
        seen: dict = {}
        for blk in fn["blocks"]:
            for ins in blk["instructions"]:
                si = ins.get("sync_info") or {}
                upd = si.get("on_update") or []
                own = {
                    u["id"]
                    for u in upd
                    if u.get("sync_type") == "semaphore"
                    and str(u.get("ant_name", "")).startswith(_ENGINE_SEM_PREFIXES)
                }
                ow = si.get("on_wait") or []
                if len(ow) >= 2:
                    kept = []
                    for w in ow:
                        if (
                            w.get("sync_type") == "semaphore"
                            and w["id"] in own
                            and w.get("wait_mode") == "sem-ge-imm"
                            and w.get("wait_value", 1 << 30)
                            <= seen.get(w["id"], 0)
                        ):
                            continue
                        kept.append(w)
                    si["on_wait"] = kept
                for u in upd:
                    if u.get("sync_type") == "semaphore" and u.get(
                        "update_mode"
                    ) in ("sem-inc", "sem-add-imm"):
                        seen[u["id"]] = seen.get(u["id"], 0) + u.get(
                            "update_value", 1
                        )
        nid = [1 << 20]
        for blk in fn["blocks"]:
            out_insts = []
            for ins in blk["instructions"]:
                si = ins.get("sync_info") or {}
                ow = si.get("on_wait") or []
                if len(ow) >= 2 and not ins.get("ins") and not ins.get("outs"):
                    for w in ow[:-1]:
                        clone = json.loads(json.dumps(ins))
                        clone["sync_info"]["on_wait"] = [w]
                        clone["sync_info"]["on_update"] = []
                        clone["name"] = f"I-{nid[0]}"
                        nid[0] += 1
                        out_insts.append(clone)
                    si["on_wait"] = [ow[-1]]
                out_insts.append(ins)
            blk["instructions"] = out_insts
    return json.dumps(m).encode()


def _build():
    nc = bass.Bass()

    inpb = nc.declare_dram_parameter("inpb", [P, F_PRJ], BF16, isOutput=False)
    maskp = nc.declare_dram_parameter("maskp", [2, 2 * SL], BF16, isOutput=False)
    out_dt = BF16 if OUT_BF16 else F32
    S_out = nc.declare_dram_parameter("S", [SL, SL], out_dt, isOutput=True)

    with tile.TileContext(nc) as tc:
        with (
            tc.tile_pool(name="const", bufs=1) as cpool,
            tc.tile_pool(name="pproj", bufs=4, space=bass.MemorySpace.PSUM) as pproj,
            tc.tile_pool(name="pacc", bufs=1, space=bass.MemorySpace.PSUM) as pacc,
            tc.tile_pool(name="pjunk", bufs=1, space=bass.MemorySpace.PSUM) as pjunk,
        ):
            mask_sb = cpool.tile([2, 2 * SL], BF16, tag="mask")
            inp_sb = cpool.tile([P, F_PRJ], BF16, tag="inp")
            tail_sb = inp_sb[:, OFF_TAIL : OFF_TAIL + 2 * T_TOT].bitcast(F32)
            bh_sb = tail_sb[:, T_BH : T_BH + DH]

            ones_f16 = cpool.tile([P, FD], F16, tag="ones")
            zero_b = cpool.tile([P, 1], F32, tag="zero_b")
            nc.vector.memset(ones_f16[:, :], 1.0)
            nc.vector.memset(zero_b[:, :], 0.0)

            # ---- PE warmup: the HAM clock gate unthrottles (1.2 -> 2.4GHz)
            # only after ~3.4us of sustained matmul activity, and M=1 junk
            # matmuls don't register as PE-busy (measured) -- use full
            # 128-row matmuls. The PE is otherwise idle while the input
            # DMAs stream in; burn that window so real matmuls run warm.
            warm_ps = pjunk.tile([P, WARM_FREE], F32, tag="warm")
            for _ in range(WARMUP_N):
                nc.tensor.matmul(
                    warm_ps[:, :], ones_f16[:, 0:P], ones_f16[:, 0:WARM_FREE],
                    start=True, stop=True, skip_group_check=True,
                )

            # ---- input DMAs: ~128KB chunks on the 3 DGE queues
            # (sync/scalar HWDGE + gpsimd SWDGE), in need-order: x + Wh-dh0
            # gate the first projection; Wc last. The tail rides
            # contiguously behind x ko6-7 on the scalar queue.
            def xcols(k0, k1, tail=False):
                lo, hi = OFF_X + k0 * SL, OFF_X + k1 * SL
                if tail:
                    hi += 2 * T_TOT
                return (inp_sb[:, lo:hi], inpb[:, lo:hi])

            def wcols(base, dh, k0, k1):
                lo = base + dh * KO * P + k0 * P
                hi = base + dh * KO * P + k1 * P
                return (inp_sb[:, lo:hi], inpb[:, lo:hi])

            for eng, (dst, src) in [
                (nc.sync, xcols(0, 2)),
                (nc.scalar, xcols(6, 8, tail=True)),
                (nc.gpsimd, wcols(OFF_WH, 0, 0, 4)),
                (nc.sync, xcols(2, 4)),
                (nc.scalar, xcols(4, 6)),
                (nc.gpsimd, wcols(OFF_WH, 0, 4, 8)),
                (nc.sync, wcols(OFF_WH, 1, 0, 4)),
                (nc.scalar, wcols(OFF_WH, 1, 4, 8)),
                (nc.gpsimd, wcols(OFF_WC, 0, 0, 4)),
                (nc.sync, wcols(OFF_WC, 0, 4, 8)),
                (nc.scalar, wcols(OFF_WC, 1, 0, 4)),
                (nc.gpsimd, wcols(OFF_WC, 1, 4, 8)),
                (nc.sync, (mask_sb[:, :], maskp[:, :])),
            ]:
                eng.dma_start(out=dst, in_=src)
            # keep-warm filler between DMA-gated projection segments: the
            # HAM clock gate re-throttles after a ~3.4us fully-idle window;
            # a couple of junk matmuls per stall keep the window busy
            def keep_warm(n):
                for _ in range(n):
                    nc.tensor.matmul(
                        warm_ps[:, :], ones_f16[:, 0:P],
                        ones_f16[:, 0:WARM_FREE],
                        start=True, stop=True, skip_group_check=True,
                    )

            # ---- absorbers: fold each DMA semaphore into consumer engines
            junk = pjunk.tile([1, 32], F32, tag="junk")
            junk_n = [0]

            def absorb_pe(ap):
                k = junk_n[0] % 32
                junk_n[0] += 1
                nc.tensor.matmul(
                    junk[:, k : k + 1], ap, ap,
                    start=True, stop=True, skip_group_check=True,
                )

            # ACT: trigger the Sin table load (~1.3us DMA on the scalar
            # ring) after the scalar engine's own chunk issues
            junk_act = cpool.tile([P, 1], F32, tag="junk_act")
            nc.scalar.activation(
                junk_act[:, :], zero_b[:, :], AF.Sin, bias=zero_b[:, 0:1],
                scale=ACT_SCALE,
            )

            # ---- projections: accumulate over ko in DMA-arrival order
            def wsl(base, dh, ko):
                lo = base + dh * KO * P + ko * P
                return inp_sb[:, lo : lo + P]

            def xsl(ko):
                return inp_sb[:, OFF_X + ko * SL : OFF_X + (ko + 1) * SL]

            ps_h = [pproj.tile([P, SL], F32, tag="proj", name=f"ps_h{i}")
                    for i in range(DH)]
            ps_c = [pproj.tile([P, SL], F32, tag="proj", name=f"ps_c{i}")
                    for i in range(DH)]

            def proj_seg(base, ps, dh, kos, n_done):
                for i, ko in enumerate(kos):
                    nc.tensor.matmul(
                        ps[dh][:, :], wsl(base, dh, ko), xsl(ko),
                        start=(n_done + i == 0), stop=(n_done + i == KO - 1),
                    )

            # head dh0 in chunk-arrival order: [0,1] (r1), [6,7] (r1),
            # [2,3] (r2), [4,5] (r2)
            absorb_pe(inp_sb[:, OFF_X : OFF_X + 1])
            absorb_pe(wsl(OFF_WH, 0, 0)[:, 0:1])
            proj_seg(OFF_WH, ps_h, 0, [0, 1], 0)
            absorb_pe(inp_sb[:, OFF_X + 6 * SL : OFF_X + 6 * SL + 1])
            proj_seg(OFF_WH, ps_h, 0, [6, 7], 2)
            absorb_pe(inp_sb[:, OFF_X + 2 * SL : OFF_X + 2 * SL + 1])
            proj_seg(OFF_WH, ps_h, 0, [2, 3], 4)
            absorb_pe(inp_sb[:, OFF_X + 4 * SL : OFF_X + 4 * SL + 1])
            absorb_pe(wsl(OFF_WH, 0, 4)[:, 0:1])
            proj_seg(OFF_WH, ps_h, 0, [4, 5], 6)
            # head dh1
            keep_warm(2)
            absorb_pe(wsl(OFF_WH, 1, 0)[:, 0:1])
            proj_seg(OFF_WH, ps_h, 1, [0, 1, 2, 3], 0)
            absorb_pe(wsl(OFF_WH, 1, 4)[:, 0:1])
            proj_seg(OFF_WH, ps_h, 1, [4, 5, 6, 7], 4)
            # child dh0, dh1
            keep_warm(2)
            absorb_pe(wsl(OFF_WC, 0, 0)[:, 0:1])
            proj_seg(OFF_WC, ps_c, 0, [0, 1, 2, 3], 0)
            absorb_pe(wsl(OFF_WC, 0, 4)[:, 0:1])
            proj_seg(OFF_WC, ps_c, 0, [4, 5, 6, 7], 4)
            keep_warm(2)
            absorb_pe(wsl(OFF_WC, 1, 0)[:, 0:1])
            proj_seg(OFF_WC, ps_c, 1, [0, 1, 2, 3], 0)
            absorb_pe(wsl(OFF_WC, 1, 4)[:, 0:1])
            proj_seg(OFF_WC, ps_c, 1, [4, 5, 6, 7], 4)

            # ================= per-side feature pipeline =================
            h_ints = cpool.tile([P, NSLOT, FD], F32, tag="int_h")
            c_ints = cpool.tile([P, NSLOT, FD], F32, tag="int_c")
            h_feats = cpool.tile([P, NSLOT, FD], F16, tag="feat_h")
            c_feats = cpool.tile([P, NSLOT, FD], F16, tag="feat_c")
            csc = cpool.tile([P, NSLOT, FD], F16, tag="csc")
            h_sb = cpool.tile([P, FD], F32, tag="v_h")
            c_sb = cpool.tile([P, FD], F32, tag="v_c")
            h_f16 = cpool.tile([P, FD], F16, tag="h16")   # w*a1*(h+bh)
            c0 = cpool.tile([P, FD], F16, tag="c0")       # w*a1*c
            junk_dve = cpool.tile([P, 1], F32, tag="junk_dve")

            def dsl(dh):
                return slice(dh * SL, (dh + 1) * SL)

            def slots(ints, v_sb, dh):
                # sin slots m, then cos slots M+m (phase pi/2 = 16384 units);
                # f32 out with the mantissa-pinning MAGIC offset
                for s, ph in [(0, 0.0), (M, QUARTER)]:
                    for m in range(M):
                        nc.vector.tensor_scalar(
                            out=ints[:, s + m, dsl(dh)],
                            in0=v_sb[:, dsl(dh)],
                            scalar1=float(OM[m] * PH_SCALE),
                            scalar2=float(ph + MAGIC),
                            op0=ALU.mult,
                            op1=ALU.add,
                        )

            def giant_sin(feats, ints, s0, s1, dh):
                nc.scalar.activation(
                    feats[:, s0:s1, dsl(dh)],
                    ints[:, s0:s1, dsl(dh)].bitcast(I16)
                    .rearrange("p s (n two) -> p s n two", two=2)[:, :, :, 0],
                    AF.Sin,
                    bias=zero_b[:, 0:1],
                    scale=ACT_SCALE,
                )

            # ---- feature pipeline, emitted in dependency order (tile
            # builds deps from emission order); engine assignment balances
            # DVE ~= ACT: ACT takes the head evicts (its pre-sin idle) and
            # the dh0 folds; DVE runs the slot chain lean so the last
            # C-side slots (which gate ACT's final sin passes) land ASAP.
            def wb_col(m, dh):
                k = T_WB + m * DH + dh
                return tail_sb[:, k : k + 1]

            nc.vector.tensor_copy(junk_dve[:, :], tail_sb[:, 0:1])  # tail->DVE
            nc.scalar.copy(junk_act[:, :], tail_sb[:, 0:1])         # tail->ACT
            # head evicts + h_f16 on ACT (its pre-sin idle window); DVE
            # runs a lean slot chain so the last C-side slots (which gate
            # ACT's final sin passes) land ASAP, then does the folds.
            nc.scalar.activation(
                h_sb[:, dsl(0)], ps_h[0][:, :], AF.Identity,
                bias=bh_sb[:, 0:1])
            slots(h_ints, h_sb, 0)
            nc.scalar.activation(
                h_sb[:, dsl(1)], ps_h[1][:, :], AF.Identity,
                bias=bh_sb[:, 1:2])
            for dh in range(DH):
                nc.scalar.activation(
                    h_f16[:, dsl(dh)], h_sb[:, dsl(dh)], AF.Copy,
                    bias=0.0, scale=tail_sb[:, T_WA1 + dh : T_WA1 + dh + 1])
            # child dh0 evict + c0 ride between the DVE slot groups
            nc.vector.tensor_copy(c_sb[:, dsl(0)], ps_c[0][:, :])
            nc.vector.tensor_scalar_mul(
                c0[:, dsl(0)], c_sb[:, dsl(0)],
                tail_sb[:, T_WA1 : T_WA1 + 1])
            slots(h_ints, h_sb, 1)
            giant_sin(h_feats, h_ints, 0, M, 0)      # sin H dh0
            giant_sin(h_feats, h_ints, M, NSLOT, 0)  # cos H dh0
            giant_sin(h_feats, h_ints, 0, M, 1)      # sin H dh1
            giant_sin(h_feats, h_ints, M, NSLOT, 1)  # cos H dh1
            slots(c_ints, c_sb, 0)
            giant_sin(c_feats, c_ints, 0, M, 0)      # sin C dh0
            giant_sin(c_feats, c_ints, M, NSLOT, 0)  # cos C dh0
            nc.vector.tensor_copy(c_sb[:, dsl(1)], ps_c[1][:, :])
            nc.vector.tensor_scalar_mul(
                c0[:, dsl(1)], c_sb[:, dsl(1)],
                tail_sb[:, T_WA1 + 1 : T_WA1 + 2])
            slots(c_ints, c_sb, 1)
            giant_sin(c_feats, c_ints, 0, M, 1)      # sin C dh1
            giant_sin(c_feats, c_ints, M, NSLOT, 1)  # cos C dh1
            # folds on DVE ordered by MM-group consumption (sin-dh0,
            # cos-dh0, sin-dh1, cos-dh1); the final group's last two go to
            # ACT, which is idle after its last sin pass
            for s_off, hs_off, dh in [(0, M, 0), (M, 0, 0), (0, M, 1)]:
                for m in range(M):
                    nc.vector.tensor_scalar_mul(
                        csc[:, s_off + m, dsl(dh)],
                        h_feats[:, hs_off + m, dsl(dh)],
                        wb_col(m, dh))
            nc.vector.tensor_scalar_mul(
                csc[:, M, dsl(1)], h_feats[:, 0, dsl(1)], wb_col(0, 1))
            for m in range(1, M):
                nc.scalar.activation(
                    csc[:, M + m, dsl(1)], h_feats[:, m, dsl(1)],
                    AF.Copy, bias=0.0, scale=wb_col(m, 1))

            # ---- the big contraction: S[i,j] += Cf^T @ Hf per chunk
            acc = [pacc.tile([P, SL], F32, tag=f"acc{i}", name=f"acc{i}")
                   for i in range(2)]

            def mm(ih, lhsT, rhs, start=False, stop=False):
                nc.tensor.matmul(
                    acc[ih][:, :], lhsT, rhs, start=start, stop=stop)

            # mask chunk first: it lands early and opens the accumulation
            absorb_pe(mask_sb[:, 0:1])
            for ih in range(2):
                nc.tensor.matmul(
                    acc[ih][:, :],
                    mask_sb[:, SL + ih * P : SL + (ih + 1) * P],
                    mask_sb[:, 0:SL],
                    start=True, stop=False,
                )

            # trig chunks follow the ACT c-side pass order: (sin dh0),
            # (cos dh0), [lin chunks], (sin dh1), (cos dh1 = final)
            def trig_absorb(s0, dh):
                absorb_pe(c_feats[:, s0 + M - 1, dh * SL : dh * SL + 1])
                absorb_pe(csc[:, s0 + M - 1, dh * SL : dh * SL + 1])

            def trig_mm(ih, s, dh, stop=False):
                mm(ih, c_feats[:, s, dh * SL + ih * P : dh * SL + (ih + 1) * P],
                   csc[:, s, dsl(dh)], stop=stop)

            # lin1: ones_i x (w*a1*h)_j ; lin2: (w*a1*c)_i x ones_j; keep-
            # warm bursts bridge the PE-idle stretch until the trig chunks'
            # features land (a >3.4us idle window would re-throttle HAM)
            absorb_pe(h_f16[:, 0:1])
            for dh in range(DH):
                for ih in range(2):
                    mm(ih, ones_f16[:, ih * P : (ih + 1) * P],
                       h_f16[:, dsl(dh)])
            keep_warm(8)
            for dh in range(DH):
                absorb_pe(c0[:, dh * SL : dh * SL + 1])
                for ih in range(2):
                    mm(ih, c0[:, dh * SL + ih * P : dh * SL + (ih + 1) * P],
                       ones_f16[:, 0:SL])
            keep_warm(4)

            for s0, dh in [(0, 0), (M, 0), (0, 1)]:
                trig_absorb(s0, dh)
                for m in range(M):
                    for ih in range(2):
                        trig_mm(ih, s0 + m, dh)

            # final group: finish ih0 first so its epilogue and output DMA
            # overlap ih1's tail
            s_t = cpool.tile([P, 2, SL], out_dt, tag="sout")
            trig_absorb(M, 1)
            for m in range(M):
                trig_mm(0, M + m, 1, stop=(m == M - 1))
            # ACT is idle after its last sin pass; DVE handles ih1
            nc.scalar.copy(s_t[:, 0, :], acc[0][:, :])
            nc.sync.dma_start(out=S_out[0:P, :], in_=s_t[:, 0, :])
            for m in range(M):
                trig_mm(1, M + m, 1, stop=(m == M - 1))
            nc.vector.tensor_copy(s_t[:, 1, :], acc[1][:, :])
            nc.scalar.dma_start(out=S_out[P : 2 * P, :], in_=s_t[:, 1, :])

    _orig = nc.to_json_bytes
    nc.to_json_bytes = lambda: _strip_self_waits(_orig())
    return nc


def _prep_in_maps(inputs):
    import ml_dtypes

    bf16 = ml_dtypes.bfloat16
    x = np.ascontiguousarray(np.asarray(inputs["encoded_text"], dtype=np.float32))
    mask = np.asarray(inputs["mask"])
    Wh = np.asarray(inputs["Wh"], dtype=np.float32)
    bh = np.asarray(inputs["bh"], dtype=np.float32)
    Wc = np.asarray(inputs["Wc"], dtype=np.float32)
    w_out = np.asarray(inputs["w_out"], dtype=np.float32)

    def pack_w(W):  # (ED, ENC) -> (P, DH*KO*P): dh-major, then ko
        Wt = W.T.reshape(KO, P, DH, P)
        return np.ascontiguousarray(
            Wt.transpose(1, 2, 0, 3).reshape(P, F_SEC)
        ).astype(bf16)

    WhS, WcS = pack_w(Wh), pack_w(Wc)
    mm = ((1.0 - mask.astype(np.float32)) * -1.0e8).astype(np.float32)  # (BS, SL)
    wdh = w_out.reshape(DH, P).T              # (P, DH): w by (dlo, dh)

    tailv = np.zeros((P, T_TOT), dtype=np.float32)
    tailv[:, T_BH : T_BH + DH] = bh.reshape(DH, P).T
    for m in range(M):
        for dh in range(DH):
            tailv[:, T_WB + m * DH + dh] = wdh[:, dh] * BM[m]
    for dh in range(DH):
        tailv[:, T_WA1 + dh] = wdh[:, dh] * A1
    tail_bf = np.ascontiguousarray(tailv).view(bf16)  # (P, 2*T_TOT) raw bytes

    in_maps = []
    for b in range(BS):
        xS = np.ascontiguousarray(
            x[b].T.reshape(KO, P, SL).transpose(1, 0, 2).reshape(P, F_SEC)
        ).astype(bf16)
        packed = np.empty((P, F_PRJ), dtype=bf16)
        packed[:, OFF_X : OFF_X + F_SEC] = xS
        packed[:, OFF_TAIL : OFF_TAIL + 2 * T_TOT] = tail_bf
        packed[:, OFF_WC : OFF_WC + F_SEC] = WcS
        packed[:, OFF_WH : OFF_WH + F_SEC] = WhS
        maskv = np.zeros((2, 2 * SL), dtype=np.float32)
        maskv[0, 0:SL] = mm[b]          # rhs row0: mm_j
        maskv[1, 0:SL] = 1.0            # rhs row1: ones
        maskv[0, SL:] = 1.0             # lhsT row0: ones (pairs with mm_j)
        maskv[1, SL:] = mm[b]           # lhsT row1: mm_i
        in_maps.append(dict(inpb=packed, maskp=maskv.astype(bf16)))
    return in_maps


def run(inputs, trace=False, **kw):
    if "nc" not in _CACHE:
        _CACHE["nc"] = _build()
    nc = _CACHE["nc"]
    in_maps = _prep_in_maps(inputs)
    res = run_bass_kernel_spmd(nc, in_maps, list(range(BS)), trace=trace, **kw)
    out = np.stack(
        [np.asarray(res.results[b]["S"], dtype=np.float32) for b in range(BS)],
        axis=0,
    )
    return out, res


def kernel(**inputs):
    return run(inputs)[0]


# revision 39
# speedup vs baseline: 1.0577x; 1.0577x over previous
"""Trainium2 Bass kernel for nn_KGEdges via low-rank trigonometric factorization.

S[b,i,j] = sum_d w[d] * tanh(h[b,j,d] + c[b,i,d]) + mm[b,i] + mm[b,j]
  with h = x@Wh.T + bh, c = x@Wc.T.

Math: tanh(x) ~= a1*x + sum_m b_m sin(om_m x) (weighted LSQ fit, rel err
~2e-3 under the data distribution), and each sine factorizes over h+c:
sin(om(h+c)) = sin(om h)cos(om c) + cos(om h)sin(om c). The (i,j) plane
then comes from ONE PE contraction over (feature, d):

  S[i,j] = sum_{f,d} Cf[f,d,i] * Hf[f,d,j]
  Hf: {1, h, sin(om_m h), cos(om_m h)}            (2M+2 features)
  Cf: {w*a1*c, 1, w*b_m cos(om_m c), w*b_m sin(om_m c)}
  (+ a 2-row mask chunk: [1;mm_i] x [mm_j;1])

Range reduction for Sin (HW spline valid on [-pi,pi] only) is done in
fixed point via an f32 mantissa trick: one fused DVE tensor_scalar per
(freq, phase, d-half) computes t = v*(om*65536/2pi) + (3*2^22 + ph),
which pins the f32 exponent so the value's *low halfword* IS
round(v*s1+ph) mod 2^16, i.e. the angle mod 2pi mapped onto
[-32768,32768) ~ [-pi,pi) (i16 stride-2 bitcast view), consumed by giant
Sin passes with scale=2pi/65536.

Perf structure (vs the 35.0us predecessor; measured 30.6-31.4us):
 - 40 full-array junk matmuls warm the PE HAM clock gate during the
   DMA-in window so real matmuls run at 2.4GHz instead of the cold
   1.2GHz (M=1 junk matmuls do NOT register as PE activity; the warm-up
   block must exceed the ~3.4us HAM window); keep-warm bursts bridge
   mid-kernel PE-idle stretches so HAM never re-throttles.
 - input DMA in ~128KB chunks on the 3 DGE queues in need-order
   (x + Wh-dh0 -> Wh-dh1 -> Wc); weights packed dh-major so the dh0
   projection (and the feature pipeline behind it) starts early. The
   queues land one chunk every ~0.45us aggregate; input completes
   ~15us in, which gates the child-side feature chain.
 - feature work balanced across ACT (head evicts, w*a1*h, 8 sin passes,
   2 tail folds) and DVE (slot chain, child evicts, w*a1*c, 10 folds),
   ordered so the C-side slots that gate ACT's last sin passes land
   ASAP.
 - output in bf16 over two queues (host upcasts), halving the tail DMA.
Data-parallel over batch: 1 core per batch element.
"""

import os
import sys

for _p in ("/opt/trn_rl_repo", "/opt/pypackages"):
    if _p not in sys.path and os.path.isdir(_p):
        sys.path.insert(0, _p)

import numpy as np

from concourse import bass, tile
import concourse.mybir as mybir
from concourse.bass_utils import run_bass_kernel_spmd

BS, SL, ENC, ED = 8, 256, 1024, 256
P = 128
KO = ENC // P      # 8 contraction chunks for projections
DH = ED // P       # 2 d-halves
FD = DH * SL       # 512: free size of one (d, s) plane per partition

# sine fit of tanh on N(0,sqrt2): tanh(x) ~= A1*x + sum b sin(om x)
OM = [0.8457601956781701, 1.7785856286560242, 2.9048194005148917]
BM = [0.4985272230495812, 0.1235633227644961, 0.026858281756312094]
M = len(OM)
A1 = 0.2644975130911238
TWO_PI = 2.0 * np.pi
PH_SCALE = 65536.0 / TWO_PI        # angle -> fixed-point units
ACT_SCALE = float(TWO_PI / 65536.0)
OFF = float(2 ** 20)               # keeps the TS output positive in f32
QUARTER = 16384.0                  # pi/2 in fixed-point units (2^16/4)
NSLOT = 2 * M                      # slots per side: sin m -> m, cos m -> M+m

# toggles for bisecting hardware failures
WARMUP_N = 40                      # junk matmuls to warm the PE clock gate
                                   # (>3.4us contiguous: the HAM window,
                                   # plus margin for window phase jitter)
WARM_FREE = 128
OUT_BF16 = True
# f32 mantissa trick: t = v*s1 + (3*2^22 + ph) pins the exponent so the f32
# mantissa's low halfword IS round(v*s1+ph) mod 2^16 -- the same angle the
# i32-convert path produced, but as a pure f32 single-src op (DVE 2x-mode
# eligible, no int convert).
MAGIC = float(3 * 2 ** 22)

# f32 tail param columns (per-partition vectors)
T_BH = 0                    # bh by d-half                     (DH cols)
T_WB = T_BH + DH            # w*b_m by (m, dh)                 (M*DH cols)
T_WA1 = T_WB + M * DH       # w*a1 by dh                       (DH cols)
T_TOT = T_WA1 + DH

# packed bf16 input: [xS | tail(f32->bf16 pairs) | WcS | WhS]; the tail rides
# inside the x ko6-7 DMA chunk so it needs no DMA (and receipt) of its own
F_SEC = KO * SL
OFF_X = 0
OFF_TAIL = F_SEC
OFF_WC = F_SEC + 2 * T_TOT
OFF_WH = OFF_WC + F_SEC
F_PRJ = OFF_WH + F_SEC

F32 = mybir.dt.float32
F16 = mybir.dt.float16
BF16 = mybir.dt.bfloat16
I32 = mybir.dt.int32
I16 = mybir.dt.int16
AF = mybir.ActivationFunctionType
ALU = mybir.AluOpType

_CACHE: dict = {}

_ENGINE_SEM_PREFIXES = ("Activation", "DVE", "PE", "Pool", "SP", "DMAHW", "DMASW")


def _strip_self_waits(raw: bytes) -> bytes:
    """Remove provably-satisfied self-engine semaphore waits; split residual
    multi-waits on operand-free sync instructions (walrus encodes at most one
    sync wait per instruction)."""
    import json

    m = json.loads(raw)
    for fn in m["functions"]:
        seen: dict = {}
        for blk in fn["blocks"]:
            for ins in blk["instructions"]:
                si = ins.get("sync_info") or {}
                upd = si.get("on_update") or []
                own = {
                    u["id"]
                    for u in upd
                    if u.get("sync_type") == "semaphore"
                    and str(u.get("ant_name", "")).startswith(_ENGINE_SEM_PREFIXES)
                }
                ow = si.get("on_wait") or []
                if len(ow) >= 2:
                    kept = []
                    for w in ow:
                        if (
                            w.get("sync_type") == "semaphore"
                            and w["id"] in own
                            and w.get("wait_mode") == "sem-ge-imm"
                            and w.get("wait_value", 1 << 30)
                            <= seen.get(w["id"], 0)
                        ):
                            continue
                        kept.append(w)
                    si["on_wait"] = kept
                for u in upd:
                    if u.get("sync_type") == "semaphore" and u.get(
                        "update_mode"
                    ) in ("sem-inc", "sem-add-imm"):
                        seen[u["id"]] = seen.get(u["id"], 0) + u.get(
                            "update_value", 1
                        )
        nid = [1 << 20]
        for blk in fn["blocks"]:
            out_insts = []
            for ins in blk["instructions"]:
                si = ins.get("sync_info") or {}
                ow = si.get("on_wait") or []
                if len(ow) >= 2 and not ins.get("ins") and not ins.get("outs"):
                    for w in ow[:-1]:
                        clone = json.loads(json.dumps(ins))
                        clone["sync_info"]["on_wait"] = [w]
                        clone["sync_info"]["on_update"] = []
                        clone["name"] = f"I-{nid[0]}"
                        nid[0] += 1
                        out_insts.append(clone)
                    si["on_wait"] = [ow[-1]]
                out_insts.append(ins)
            blk["instructions"] = out_insts
    return json.dumps(m).encode()


def _build():
    nc = bass.Bass()

    inpb = nc.declare_dram_parameter("inpb", [P, F_PRJ], BF16, isOutput=False)
    maskp = nc.declare_dram_parameter("maskp", [2, 2 * SL], BF16, isOutput=False)
    out_dt = BF16 if OUT_BF16 else F32
    S_out = nc.declare_dram_parameter("S", [SL, SL], out_dt, isOutput=True)

    with tile.TileContext(nc) as tc:
        with (
            tc.tile_pool(name="const", bufs=1) as cpool,
            tc.tile_pool(name="pproj", bufs=4, space=bass.MemorySpace.PSUM) as pproj,
            tc.tile_pool(name="pacc", bufs=1, space=bass.MemorySpace.PSUM) as pacc,
            tc.tile_pool(name="pjunk", bufs=1, space=bass.MemorySpace.PSUM) as pjunk,
        ):
            mask_sb = cpool.tile([2, 2 * SL], BF16, tag="mask")
            inp_sb = cpool.tile([P, F_PRJ], BF16, tag="inp")
            tail_sb = inp_sb[:, OFF_TAIL : OFF_TAIL + 2 * T_TOT].bitcast(F32)
            bh_sb = tail_sb[:, T_BH : T_BH + DH]

            ones_f16 = cpool.tile([P, FD], F16, tag="ones")
            zero_b = cpool.tile([P, 1], F32, tag="zero_b")
            nc.vector.memset(ones_f16[:, :], 1.0)
            nc.vector.memset(zero_b[:, :], 0.0)

            # ---- PE warmup: the HAM clock gate unthrottles (1.2 -> 2.4GHz)
            # only after ~3.4us of sustained matmul activity, and M=1 junk
            # matmuls don't register as PE-busy (measured) -- use full
            # 128-row matmuls. The PE is otherwise idle while the input
            # DMAs stream in; burn that window so real matmuls run warm.
            warm_ps = pjunk.tile([P, WARM_FREE], F32, tag="warm")
            for _ in range(WARMUP_N):
                nc.tensor.matmul(
                    warm_ps[:, :], ones_f16[:, 0:P], ones_f16[:, 0:WARM_FREE],
                    start=True, stop=True, skip_group_check=True,
                )

            # ---- input DMAs: ~128KB chunks on the 3 DGE queues
            # (sync/scalar HWDGE + gpsimd SWDGE), in need-order: x + Wh-dh0
            # gate the first projection; Wc last. The tail rides
            # contiguously behind x ko6-7 on the scalar queue.
            def xcols(k0, k1, tail=False):
                lo, hi = OFF_X + k0 * SL, OFF_X + k1 * SL
                if tail:
                    hi += 2 * T_TOT
                return (inp_sb[:, lo:hi], inpb[:, lo:hi])

            def wcols(base, dh, k0, k1):
                lo = base + dh * KO * P + k0 * P
                hi = base + dh * KO * P + k1 * P
                return (inp_sb[:, lo:hi], inpb[:, lo:hi])

            for eng, (dst, src) in [
                (nc.sync, xcols(0, 2)),
                (nc.scalar, xcols(6, 8, tail=True)),
                (nc.gpsimd, wcols(OFF_WH, 0, 0, 8)),
                (nc.sync, xcols(2, 4)),
                (nc.scalar, xcols(4, 6)),
                (nc.sync, wcols(OFF_WH, 1, 0, 4)),
                (nc.scalar, wcols(OFF_WH, 1, 4, 8)),
                (nc.gpsimd, wcols(OFF_WC, 0, 0, 4)),
                (nc.sync, wcols(OFF_WC, 0, 4, 8)),
                (nc.scalar, wcols(OFF_WC, 1, 0, 4)),
                (nc.gpsimd, wcols(OFF_WC, 1, 4, 8)),
                (nc.sync, (mask_sb[:, :], maskp[:, :])),
            ]:
                eng.dma_start(out=dst, in_=src)
            # keep-warm filler between DMA-gated projection segments: the
            # HAM clock gate re-throttles after a ~3.4us fully-idle window;
            # a couple of junk matmuls per stall keep the window busy
            def keep_warm(n):
                for _ in range(n):
                    nc.tensor.matmul(
                        warm_ps[:, :], ones_f16[:, 0:P],
                        ones_f16[:, 0:WARM_FREE],
                        start=True, stop=True, skip_group_check=True,
                    )

            # ---- absorbers: fold each DMA semaphore into consumer engines
            junk = pjunk.tile([1, 32], F32, tag="junk")
            junk_n = [0]

            def absorb_pe(ap):
                k = junk_n[0] % 32
                junk_n[0] += 1
                nc.tensor.matmul(
                    junk[:, k : k + 1], ap, ap,
                    start=True, stop=True, skip_group_check=True,
                )

            # ACT: trigger the Sin table load (~1.3us DMA on the scalar
            # ring) after the scalar engine's own chunk issues
            junk_act = cpool.tile([P, 1], F32, tag="junk_act")
            nc.scalar.activation(
                junk_act[:, :], zero_b[:, :], AF.Sin, bias=zero_b[:, 0:1],
                scale=ACT_SCALE,
            )

            # ---- projections: accumulate over ko in DMA-arrival order
            def wsl(base, dh, ko):
                lo = base + dh * KO * P + ko * P
                return inp_sb[:, lo : lo + P]

            def xsl(ko):
                return inp_sb[:, OFF_X + ko * SL : OFF_X + (ko + 1) * SL]

            ps_h = [pproj.tile([P, SL], F32, tag="proj", name=f"ps_h{i}")
                    for i in range(DH)]
            ps_c = [pproj.tile([P, SL], F32, tag="proj", name=f"ps_c{i}")
                    for i in range(DH)]

            def proj_seg(base, ps, dh, kos, n_done):
                for i, ko in enumerate(kos):
                    nc.tensor.matmul(
                        ps[dh][:, :], wsl(base, dh, ko), xsl(ko),
                        start=(n_done + i == 0), stop=(n_done + i == KO - 1),
                    )

            # head dh0 in chunk-arrival order: [0,1] (r1), [6,7] (r1),
            # [2,3] (r2), [4,5] (r2). Only the first segment needs an
            # absorb (two fresh DMA sems at one matmul); every later
            # segment introduces at most one new sem, which its first
            # matmul can carry directly -- extra absorbs just slow the
            # PE stream (~164ns each), which now gates the child evicts.
            absorb_pe(inp_sb[:, OFF_X : OFF_X + 1])
            proj_seg(OFF_WH, ps_h, 0, [0, 1], 0)
            proj_seg(OFF_WH, ps_h, 0, [6, 7], 2)
            proj_seg(OFF_WH, ps_h, 0, [2, 3], 4)
            proj_seg(OFF_WH, ps_h, 0, [4, 5], 6)
            # head dh1
            proj_seg(OFF_WH, ps_h, 1, [0, 1, 2, 3], 0)
            proj_seg(OFF_WH, ps_h, 1, [4, 5, 6, 7], 4)
            # child dh0, dh1
            proj_seg(OFF_WC, ps_c, 0, [0, 1, 2, 3], 0)
            proj_seg(OFF_WC, ps_c, 0, [4, 5, 6, 7], 4)
            proj_seg(OFF_WC, ps_c, 1, [0, 1, 2, 3], 0)
            proj_seg(OFF_WC, ps_c, 1, [4, 5, 6, 7], 4)

            # ================= per-side feature pipeline =================
            h_ints = cpool.tile([P, NSLOT, FD], F32, tag="int_h")
            c_ints = cpool.tile([P, NSLOT, FD], F32, tag="int_c")
            h_feats = cpool.tile([P, NSLOT, FD], F16, tag="feat_h")
            c_feats = cpool.tile([P, NSLOT, FD], F16, tag="feat_c")
            csc = cpool.tile([P, NSLOT, FD], F16, tag="csc")
            h_sb = cpool.tile([P, FD], F32, tag="v_h")
            c_sb = cpool.tile([P, FD], F32, tag="v_c")
            h_f16 = cpool.tile([P, FD], F16, tag="h16")   # w*a1*(h+bh)
            c0 = cpool.tile([P, FD], F16, tag="c0")       # w*a1*c
            junk_dve = cpool.tile([P, 1], F32, tag="junk_dve")

            def dsl(dh):
                return slice(dh * SL, (dh + 1) * SL)

            def slots(ints, v_sb, dh):
                # sin slots m, then cos slots M+m (phase pi/2 = 16384 units);
                # f32 out with the mantissa-pinning MAGIC offset
                for s, ph in [(0, 0.0), (M, QUARTER)]:
                    for m in range(M):
                        nc.vector.tensor_scalar(
                            out=ints[:, s + m, dsl(dh)],
                            in0=v_sb[:, dsl(dh)],
                            scalar1=float(OM[m] * PH_SCALE),
                            scalar2=float(ph + MAGIC),
                            op0=ALU.mult,
                            op1=ALU.add,
                        )

            def giant_sin(feats, ints, s0, s1, dh):
                nc.scalar.activation(
                    feats[:, s0:s1, dsl(dh)],
                    ints[:, s0:s1, dsl(dh)].bitcast(I16)
                    .rearrange("p s (n two) -> p s n two", two=2)[:, :, :, 0],
                    AF.Sin,
                    bias=zero_b[:, 0:1],
                    scale=ACT_SCALE,
                )

            # ---- feature pipeline, emitted in dependency order (tile
            # builds deps from emission order); engine assignment balances
            # DVE ~= ACT: ACT takes the head evicts (its pre-sin idle) and
            # the dh0 folds; DVE runs the slot chain lean so the last
            # C-side slots (which gate ACT's final sin passes) land ASAP.
            def wb_col(m, dh):
                k = T_WB + m * DH + dh
                return tail_sb[:, k : k + 1]

            nc.vector.tensor_copy(junk_dve[:, :], tail_sb[:, 0:1])  # tail->DVE
            nc.scalar.copy(junk_act[:, :], tail_sb[:, 0:1])         # tail->ACT
            # head evicts + h_f16 on ACT (its pre-sin idle window); DVE
            # runs a lean slot chain so the last C-side slots (which gate
            # ACT's final sin passes) land ASAP, then does the folds.
            nc.scalar.activation(
                h_sb[:, dsl(0)], ps_h[0][:, :], AF.Identity,
                bias=bh_sb[:, 0:1])
            slots(h_ints, h_sb, 0)
            nc.scalar.activation(
                h_sb[:, dsl(1)], ps_h[1][:, :], AF.Identity,
                bias=bh_sb[:, 1:2])
            for dh in range(DH):
                nc.scalar.activation(
                    h_f16[:, dsl(dh)], h_sb[:, dsl(dh)], AF.Copy,
                    bias=0.0, scale=tail_sb[:, T_WA1 + dh : T_WA1 + dh + 1])
            # child dh0 evict + c0 ride between the DVE slot groups
            nc.vector.tensor_copy(c_sb[:, dsl(0)], ps_c[0][:, :])
            nc.vector.tensor_scalar_mul(
                c0[:, dsl(0)], c_sb[:, dsl(0)],
                tail_sb[:, T_WA1 : T_WA1 + 1])
            slots(h_ints, h_sb, 1)
            giant_sin(h_feats, h_ints, 0, M, 0)      # sin H dh0
            giant_sin(h_feats, h_ints, M, NSLOT, 0)  # cos H dh0
            giant_sin(h_feats, h_ints, 0, M, 1)      # sin H dh1
            giant_sin(h_feats, h_ints, M, NSLOT, 1)  # cos H dh1
            slots(c_ints, c_sb, 0)
            giant_sin(c_feats, c_ints, 0, M, 0)      # sin C dh0
            giant_sin(c_feats, c_ints, M, NSLOT, 0)  # cos C dh0
            nc.vector.tensor_copy(c_sb[:, dsl(1)], ps_c[1][:, :])
            nc.vector.tensor_scalar_mul(
                c0[:, dsl(1)], c_sb[:, dsl(1)],
                tail_sb[:, T_WA1 + 1 : T_WA1 + 2])
            slots(c_ints, c_sb, 1)
            giant_sin(c_feats, c_ints, 0, M, 1)      # sin C dh1
            giant_sin(c_feats, c_ints, M, NSLOT, 1)  # cos C dh1
            # folds on DVE ordered by MM-group consumption (sin-dh0,
            # cos-dh0, sin-dh1, cos-dh1); the final group's last two go to
            # ACT, which is idle after its last sin pass
            for s_off, hs_off, dh in [(0, M, 0), (M, 0, 0), (0, M, 1)]:
                for m in range(M):
                    nc.vector.tensor_scalar_mul(
                        csc[:, s_off + m, dsl(dh)],
                        h_feats[:, hs_off + m, dsl(dh)],
                        wb_col(m, dh))
            nc.vector.tensor_scalar_mul(
                csc[:, M, dsl(1)], h_feats[:, 0, dsl(1)], wb_col(0, 1))
            for m in range(1, M):
                nc.scalar.activation(
                    csc[:, M + m, dsl(1)], h_feats[:, m, dsl(1)],
                    AF.Copy, bias=0.0, scale=wb_col(m, 1))

            # ---- the big contraction: S[i,j] += Cf^T @ Hf per chunk
            acc = [pacc.tile([P, SL], F32, tag=f"acc{i}", name=f"acc{i}")
                   for i in range(2)]

            def mm(ih, lhsT, rhs, start=False, stop=False):
                nc.tensor.matmul(
                    acc[ih][:, :], lhsT, rhs, start=start, stop=stop)

            # mask chunk first: it lands early and opens the accumulation
            absorb_pe(mask_sb[:, 0:1])
            for ih in range(2):
                nc.tensor.matmul(
                    acc[ih][:, :],
                    mask_sb[:, SL + ih * P : SL + (ih + 1) * P],
                    mask_sb[:, 0:SL],
                    start=True, stop=False,
                )

            # trig chunks follow the ACT c-side pass order: (sin dh0),
            # (cos dh0), [lin chunks], (sin dh1), (cos dh1 = final)
            def trig_absorb(s0, dh):
                absorb_pe(c_feats[:, s0 + M - 1, dh * SL : dh * SL + 1])
                absorb_pe(csc[:, s0 + M - 1, dh * SL : dh * SL + 1])

            def trig_mm(ih, s, dh, stop=False):
                mm(ih, c_feats[:, s, dh * SL + ih * P : dh * SL + (ih + 1) * P],
                   csc[:, s, dsl(dh)], stop=stop)

            # lin1: ones_i x (w*a1*h)_j ; lin2: (w*a1*c)_i x ones_j; keep-
            # warm bursts bridge the PE-idle stretch until the trig chunks'
            # features land (a >3.4us idle window would re-throttle HAM)
            absorb_pe(h_f16[:, 0:1])
            for dh in range(DH):
                for ih in range(2):
                    mm(ih, ones_f16[:, ih * P : (ih + 1) * P],
                       h_f16[:, dsl(dh)])
            keep_warm(8)
            for dh in range(DH):
                absorb_pe(c0[:, dh * SL : dh * SL + 1])
                for ih in range(2):
                    mm(ih, c0[:, dh * SL + ih * P : dh * SL + (ih + 1) * P],
                       ones_f16[:, 0:SL])
            keep_warm(4)

            for s0, dh in [(0, 0), (M, 0), (0, 1)]:
                trig_absorb(s0, dh)
                for m in range(M):
                    for ih in range(2):
                        trig_mm(ih, s0 + m, dh)

            # final group: finish ih0 first so its epilogue and output DMA
            # overlap ih1's tail
            s_t = cpool.tile([P, 2, SL], out_dt, tag="sout")
            trig_absorb(M, 1)
            for m in range(M):
                trig_mm(0, M + m, 1, stop=(m == M - 1))
            # ACT is idle after its last sin pass; DVE handles ih1
            nc.scalar.copy(s_t[:, 0, :], acc[0][:, :])
            nc.sync.dma_start(out=S_out[0:P, :], in_=s_t[:, 0, :])
            for m in range(M):
                trig_mm(1, M + m, 1, stop=(m == M - 1))
            nc.vector.tensor_copy(s_t[:, 1, :], acc[1][:, :])
            nc.scalar.dma_start(out=S_out[P : 2 * P, :], in_=s_t[:, 1, :])

    _orig = nc.to_json_bytes
    nc.to_json_bytes = lambda: _strip_self_waits(_orig())
    return nc


def _prep_in_maps(inputs):
    import ml_dtypes

    bf16 = ml_dtypes.bfloat16
    x = np.ascontiguousarray(np.asarray(inputs["encoded_text"], dtype=np.float32))
    mask = np.asarray(inputs["mask"])
    Wh = np.asarray(inputs["Wh"], dtype=np.float32)
    bh = np.asarray(inputs["bh"], dtype=np.float32)
    Wc = np.asarray(inputs["Wc"], dtype=np.float32)
    w_out = np.asarray(inputs["w_out"], dtype=np.float32)

    def pack_w(W):  # (ED, ENC) -> (P, DH*KO*P): dh-major, then ko
        Wt = W.T.reshape(KO, P, DH, P)
        return np.ascontiguousarray(
            Wt.transpose(1, 2, 0, 3).reshape(P, F_SEC)
        ).astype(bf16)

    WhS, WcS = pack_w(Wh), pack_w(Wc)
    mm = ((1.0 - mask.astype(np.float32)) * -1.0e8).astype(np.float32)  # (BS, SL)
    wdh = w_out.reshape(DH, P).T              # (P, DH): w by (dlo, dh)

    tailv = np.zeros((P, T_TOT), dtype=np.float32)
    tailv[:, T_BH : T_BH + DH] = bh.reshape(DH, P).T
    for m in range(M):
        for dh in range(DH):
            tailv[:, T_WB + m * DH + dh] = wdh[:, dh] * BM[m]
    for dh in range(DH):
        tailv[:, T_WA1 + dh] = wdh[:, dh] * A1
    tail_bf = np.ascontiguousarray(tailv).view(bf16)  # (P, 2*T_TOT) raw bytes

    in_maps = []
    for b in range(BS):
        xS = np.ascontiguousarray(
            x[b].T.reshape(KO, P, SL).transpose(1, 0, 2).reshape(P, F_SEC)
        ).astype(bf16)
        packed = np.empty((P, F_PRJ), dtype=bf16)
        packed[:, OFF_X : OFF_X + F_SEC] = xS
        packed[:, OFF_TAIL : OFF_TAIL + 2 * T_TOT] = tail_bf
        packed[:, OFF_WC : OFF_WC + F_SEC] = WcS
        packed[:, OFF_WH : OFF_WH + F_SEC] = WhS
        maskv = np.zeros((2, 2 * SL), dtype=np.float32)
        maskv[0, 0:SL] = mm[b]          # rhs row0: mm_j
        maskv[1, 0:SL] = 1.0            # rhs row1: ones
        maskv[0, SL:] = 1.0             # lhsT row0: ones (pairs with mm_j)
        maskv[1, SL:] = mm[b]           # lhsT row1: mm_i
        in_maps.append(dict(inpb=packed, maskp=maskv.astype(bf16)))
    return in_maps


def run(inputs, trace=False, **kw):
    if "nc" not in _CACHE:
        _CACHE["nc"] = _build()
    nc = _CACHE["nc"]
    in_maps = _prep_in_maps(inputs)
    res = run_bass_kernel_spmd(nc, in_maps, list(range(BS)), trace=trace, **kw)
    out = np.stack(
        [np.asarray(res.results[b]["S"], dtype=np.float32) for b in range(BS)],
        axis=0,
    )
    return out, res


def kernel(**inputs):
    return run(inputs)[0]
